# revision 28
# baseline (speedup 1.0000x reference)
"""Trainium2 Bass kernel for nn_DistanceProbe.

Computes, for batch [B=8, S=2048, H=768] and proj [H=768, R=768]:
    t  = batch @ proj                      # [B, S, R]
    d2 = relu(||t_i||^2 + ||t_j||^2 - 2 t_i . t_j)   # [B, S, S]

Sharding: data-parallel over B across the 8 NeuronCores (one batch
element per core).

Numerics/performance strategy (validated vs reference in fp8 numpy sim,
max-abs/scale err ~1.3e-2 < 2e-2 gate):
  * Host splits each input into hi/lo fp8e4 pairs: x ~= xh + xl,
    proj ~= ph + pl (residual quantization, ~0.2% relative).
  * Projection t' = xh@ph + xl@ph + xh@pl on PE as fp8e4 DoubleRow
    matmuls (0.5 cyc/row: 2x bf16 rate). Dropped xl@pl term ~0.1%.
  * t' is quantized to fp8e4 (q) by the ACT engine; the SxS Gram matrix
    dots = q.T q runs as fp8e4 DoubleRow matmuls.
  * sq_i = dots_ii is read out of the diagonal-containing Gram tiles
    (identity mask + free-axis reduce on DVE; ones-matmul rebroadcast
    for the row form) => bitwise-consistent with dots, so the relu
    clamp and the zero diagonal are exact in fp8 arithmetic.
  * Epilogue relu(-2*dots + sq_j + sq_i) is two elementwise passes
    (scalar_tensor_tensor; then +bias relu) distributed across
    DVE/ACT/Pool; output written bf16 (lossless host upcast to f32).
  * Emission is chunk-pipelined: Gram wave c is interleaved one chunk
    behind the projection matmuls so every engine streams.

`reps` repeats the whole body inside one NEFF (used by test.py to
measure steady-state HW time by differencing two rep counts).
"""

import numpy as np
import ml_dtypes

import concourse.bass as bass
import concourse.tile as tile
from concourse import bacc
from concourse import masks
from concourse import mybir
from concourse.alu_op_type import AluOpType
from concourse.bass_utils import run_bass_kernel_spmd

B, S, H, R = 8, 2048, 768, 768
N_CORES = 8
P = 128          # SBUF partitions
NC_ = 512        # matmul moving free dim (one PSUM bank of fp32)
HT = H // P      # 6  k-tiles over H
RT = R // P      # 6  k-tiles over R
IT = S // P      # 16 output row tiles
SC = S // NC_    # 4  512-wide column chunks
TPC = NC_ // P   # 4  row tiles per chunk
PAIRS = HT // 2  # 3  DoubleRow k-tile pairs per 768 contraction

F32 = mybir.dt.float32
F32R = mybir.dt.float32r
BF16 = mybir.dt.bfloat16
F8 = mybir.dt.float8e4
DR = mybir.MatmulPerfMode.DoubleRow

NPF8 = ml_dtypes.float8_e4m3


def build_nc(reps=1):
    nc = bacc.Bacc("TRN2", target_bir_lowering=False, debug=False,
                   num_devices=N_CORES)

    xh_d = nc.dram_tensor("xh", [P, HT, S], F8, kind="ExternalInput")
    xl_d = nc.dram_tensor("xl", [P, HT, S], F8, kind="ExternalInput")
    ph_d = nc.dram_tensor("ph", [P, HT, R], F8, kind="ExternalInput")
    pl_d = nc.dram_tensor("pl", [P, HT, R], F8, kind="ExternalInput")
    # partition-major output layout: element (p, it, s) = d2[it*128+p, s].
    # Lets consecutive row-tile blocks share one DMA (host untiles).
    out_d = nc.dram_tensor("out", [P, IT, S], BF16, kind="ExternalOutput")

    with tile.TileContext(nc) as tc:
        with tc.tile_pool(name="persist", bufs=1) as sb, \
             tc.tile_pool(name="stg", bufs=4) as stg, \
             tc.tile_pool(name="pmm", bufs=3, space="PSUM") as pmm, \
             tc.tile_pool(name="pd", bufs=5, space="PSUM") as pdp:

            # input + qq buffers are double-buffered by rep parity so the
            # next rep's projection phase (and its input DMAs) can overlap
            # this rep's Gram waves without write-after-read hazards.
            xh_sb = [sb.tile([P, HT, S], F8, name=f"xh{i}", tag=f"xh{i}")
                     for i in range(2)]
            xl_sb = [sb.tile([P, HT, S], F8, name=f"xl{i}", tag=f"xl{i}")
                     for i in range(2)]
            ph_sb = [sb.tile([P, HT, R], F8, name=f"ph{i}", tag=f"ph{i}")
                     for i in range(2)]
            pl_sb = [sb.tile([P, HT, R], F8, name=f"pl{i}", tag=f"pl{i}")
                     for i in range(2)]
            qq_sb = [sb.tile([P, RT, S], F8, name=f"qq{i}", tag=f"qq{i}")
                     for i in range(2)]
            sqj = sb.tile([P, S], F32, name="sqj", tag="sqj")
            sqcol = sb.tile([P, IT], F32, name="sqcol", tag="sqcol")
            ident4 = sb.tile([P, NC_], F32, name="ident4", tag="id4")
            onesf = sb.tile([P, P], F32, name="onesf", tag="onesf")
            onesr = sb.tile([P, P], F32R, name="onesr", tag="onesr")

            for k in range(TPC):
                masks.make_identity(nc, ident4[:, k * P:(k + 1) * P])
            nc.vector.memset(onesf[:], 1.0)
            nc.vector.tensor_copy(onesr[:], onesf[:])

            def emit_loads(par):
                """Input DMAs for the buffers of rep parity `par`."""
                nc.sync.dma_start(ph_sb[par][:], ph_d[:, :, :])
                nc.sync.dma_start(pl_sb[par][:], pl_d[:, :, :])
                for c in range(SC):
                    cs = slice(c * NC_, (c + 1) * NC_)
                    nc.sync.dma_start(xh_sb[par][:, :, cs], xh_d[:, :, cs])
                    nc.sync.dma_start(xl_sb[par][:, :, cs], xl_d[:, :, cs])

            def emit_body(par):
                xh, xl = xh_sb[par], xl_sb[par]
                ph, pl = ph_sb[par], pl_sb[par]
                qq = qq_sb[par]

                # Epilogue engine schedule, per unit index mod 16. GPSIMD
                # cannot read PSUM, so PSUM-input stt lives on DVE; for
                # "type-B" slots ACT first evacuates -2*pd to SBUF so Pool
                # can add sqj, and the cheap bf16 relu goes to DVE.
                TYPEB = frozenset((1, 4, 7, 10, 13))
                RELU_A = frozenset((0, 2, 6, 9, 12, 14))
                unit_idx = [0]

                def emit_mm(it, jc):
                    """Gram matmul group for one [128, 512] tile."""
                    js = slice(jc * NC_, (jc + 1) * NC_)
                    pd = pdp.tile([P, NC_], F32, name="pd", tag="pd")
                    for p in range(PAIRS):
                        nc.tensor.matmul(
                            pd[:],
                            qq[:, 2 * p:2 * p + 2, it * P:(it + 1) * P],
                            qq[:, 2 * p:2 * p + 2, js],
                            start=(p == 0), stop=(p == PAIRS - 1),
                            perf_mode=DR)
                    return pd

                def emit_epilogue(it, jc, pd, dst):
                    """relu(-2*pd + sq_j + sq_i) -> bf16 into dst AP."""
                    js = slice(jc * NC_, (jc + 1) * NC_)
                    u = unit_idx[0] % 16
                    unit_idx[0] += 1
                    st = stg.tile([P, NC_], F32, name="st", tag="st",
                                  bufs=4)
                    if u in TYPEB:
                        pb = stg.tile([P, NC_], F32, name="pb", tag="pb",
                                      bufs=3)
                        nc.scalar.activation(
                            pb[:], pd[:], mybir.ActivationFunctionType.Copy,
                            bias=0.0, scale=-2.0)
                        nc.gpsimd.tensor_tensor(st[:], pb[:], sqj[:, js],
                                                AluOpType.add)
                        nc.vector.tensor_scalar(
                            dst, st[:], sqcol[:, it:it + 1], 0.0,
                            AluOpType.add, AluOpType.max)
                    else:
                        nc.vector.scalar_tensor_tensor(
                            st[:], pd[:], -2.0, sqj[:, js],
                            AluOpType.mult, AluOpType.add)
                        if u in RELU_A:
                            nc.scalar.activation(
                                dst, st[:],
                                mybir.ActivationFunctionType.Relu,
                                bias=sqcol[:, it:it + 1], scale=1.0)
                        else:
                            nc.gpsimd.tensor_scalar(
                                dst, st[:], sqcol[:, it:it + 1], 0.0,
                                AluOpType.add, AluOpType.max)

                def emit_proj_chunk(c):
                    """t' for columns chunk c -> quantized qq chunk."""
                    cs = slice(c * NC_, (c + 1) * NC_)
                    for rt in range(RT):
                        pt = pmm.tile([P, NC_], F32, name="pt", tag="pt")
                        first = True
                        for pj, xx in ((ph, xh), (ph, xl), (pl, xh)):
                            for p in range(PAIRS):
                                nc.tensor.matmul(
                                    pt[:],
                                    pj[:, 2 * p:2 * p + 2,
                                       rt * P:(rt + 1) * P],
                                    xx[:, 2 * p:2 * p + 2, cs],
                                    start=first,
                                    stop=(pj is pl and p == PAIRS - 1),
                                    perf_mode=DR)
                                first = False
                        # ACT is saturated early in the rep (deferred-tail
                        # relus + quants); chunk 0's quantize goes to DVE
                        if c == 0:
                            nc.vector.tensor_copy(qq[:, rt, cs], pt[:])
                        else:
                            nc.scalar.copy(qq[:, rt, cs], pt[:])

                def emit_wave(c):
                    """All Gram tiles with max(row_chunk, col_chunk)==c.

                    The 4 diagonal-containing tiles go first; sq for
                    chunk c is extracted from their PSUM (fused mask +
                    free-axis reduce) before any wave-c epilogue runs.
                    Row tiles of chunk c accumulate into one bf16 strip
                    so each row needs a single output DMA.
                    """
                    cs = slice(c * NC_, (c + 1) * NC_)
                    diag_pds = []
                    for k in range(TPC):
                        it = c * TPC + k
                        diag_pds.append((it, emit_mm(it, c)))
                    dm = stg.tile([P, NC_], F32R, name="dm", tag="dm",
                                  bufs=2)
                    for k, (it, pd) in enumerate(diag_pds):
                        ks = slice(k * P, (k + 1) * P)
                        nc.vector.tensor_mul(dm[:, ks], pd[:, ks],
                                             ident4[:, ks])
                    sq_ps = pmm.tile([P, NC_], F32, name="sqps", tag="pt")
                    nc.tensor.matmul(sq_ps[:], onesr[:], dm[:],
                                     start=True, stop=True)
                    nc.scalar.copy(sqj[:, cs], sq_ps[:])
                    for k, (it, pd) in enumerate(diag_pds):
                        ks = slice(k * P, (k + 1) * P)
                        nc.vector.tensor_reduce(
                            sqcol[:, it:it + 1], dm[:, ks],
                            axis=mybir.AxisListType.X, op=AluOpType.add)
                    w = (c + 1) * NC_
                    for k in range(TPC):
                        it = c * TPC + k
                        strip = stg.tile([P, S], BF16, name="rs", tag="rs",
                                         bufs=5)
                        # diag epilogue first: frees its PSUM bank before
                        # the row's remaining matmuls need banks
                        emit_epilogue(it, c, diag_pds[k][1],
                                      strip[:, c * NC_:(c + 1) * NC_])
                        for jc in range(c):
                            emit_epilogue(it, jc, emit_mm(it, jc),
                                          strip[:, jc * NC_:(jc + 1) * NC_])
                        nc.sync.dma_start(out_d[:, it, 0:w], strip[:, 0:w])
                    # column tiles (rows from earlier chunks), DMA'd in
                    # batches of 4 row-tiles via the partition-major layout
                    for it0 in range(0, c * TPC, TPC):
                        n = min(TPC, c * TPC - it0)
                        cb = stg.tile([P, TPC, NC_], BF16, name="cb",
                                      tag="cb", bufs=3)
                        for k in range(n):
                            emit_epilogue(it0 + k, c, emit_mm(it0 + k, c),
                                          cb[:, k, :])
                        nc.sync.dma_start(out_d[:, it0:it0 + n, cs],
                                          cb[:, 0:n, :])

                # chunk-pipelined schedule: wave c is emitted after
                # projection chunk c+1 so the fp8 quantize of chunk c has
                # drained before PE reaches wave c's matmuls. The next
                # rep's input DMAs are emitted once this rep's projection
                # has consumed its inputs, so their transfers overlap the
                # Gram waves.
                emit_proj_chunk(0)
                for c in range(SC):
                    if c + 1 < SC:
                        emit_proj_chunk(c + 1)
                    if c == 0:
                        emit_loads(1 - par)
                    emit_wave(c)

            emit_loads(0)
            for r in range(reps):
                emit_body(r % 2)

    nc.finalize()
    return nc


_NC_CACHE = {}


def get_nc(reps=1):
    key = reps
    if key not in _NC_CACHE:
        _NC_CACHE[key] = build_nc(reps)
    return _NC_CACHE[key]


def _split8(a):
    """hi/lo fp8e4 residual split of a float32 array."""
    hi = a.astype(NPF8)
    lo = (a - hi.astype(np.float32)).astype(NPF8)
    return hi, lo


def _pack(a8):
    """[H, N] -> [128, HT, N] partition-major tiling."""
    n = a8.shape[1]
    return np.ascontiguousarray(
        a8.reshape(HT, P, n).transpose(1, 0, 2))


def make_in_maps(batch, proj):
    ph, pl = _split8(np.ascontiguousarray(proj, dtype=np.float32))
    ph, pl = _pack(ph), _pack(pl)
    maps = []
    for b in range(B):
        xT = np.ascontiguousarray(batch[b].T, dtype=np.float32)
        xh, xl = _split8(xT)
        maps.append({"xh": _pack(xh), "xl": _pack(xl), "ph": ph, "pl": pl})
    return maps


def kernel(batch, proj):
    assert batch.shape == (B, S, H) and proj.shape == (H, R)
    nc = get_nc()
    in_maps = make_in_maps(batch, proj)
    res = run_bass_kernel_spmd(nc, in_maps, core_ids=list(range(N_CORES)))
    out = np.stack(
        [np.asarray(res.results[b]["out"]).transpose(1, 0, 2).reshape(S, S)
         for b in range(B)], axis=0)
    return out.astype(np.float32)


# revision 29
# speedup vs baseline: 1.1814x; 1.1814x over previous
"""Trainium2 Bass kernel for nn_DistanceProbe.

Computes, for batch [B=8, S=2048, H=768] and proj [H=768, R=768]:
    t  = batch @ proj                      # [B, S, R]
    d2 = relu(||t_i||^2 + ||t_j||^2 - 2 t_i . t_j)   # [B, S, S]

Sharding: data-parallel over B across the 8 NeuronCores (one batch
element per core).

Numerics/performance strategy (validated vs reference in fp8 numpy sim,
max-abs/scale err ~1.3e-2 < 2e-2 gate):
  * Host splits each input into hi/lo fp8e4 pairs: x ~= xh + xl,
    proj ~= ph + pl (residual quantization, ~0.2% relative).
  * Projection t' = xh@ph + xl@ph + xh@pl on PE as fp8e4 DoubleRow
    matmuls (0.5 cyc/row: 2x bf16 rate). Dropped xl@pl term ~0.1%.
  * t' is quantized to fp8e4 (q) by the ACT engine; the SxS Gram matrix
    dots = q.T q runs as fp8e4 DoubleRow matmuls.
  * sq_i = dots_ii is read out of the diagonal-containing Gram tiles
    (identity mask + free-axis reduce on DVE; ones-matmul rebroadcast
    for the row form) => bitwise-consistent with dots, so the relu
    clamp and the zero diagonal are exact in fp8 arithmetic.
  * Epilogue relu(-2*dots + sq_j + sq_i) is two elementwise passes
    (scalar_tensor_tensor; then +bias relu) distributed across
    DVE/ACT/Pool; output written bf16 (lossless host upcast to f32).
  * Emission is chunk-pipelined: Gram wave c is interleaved one chunk
    behind the projection matmuls so every engine streams.

`reps` repeats the whole body inside one NEFF (used by test.py to
measure steady-state HW time by differencing two rep counts).
"""

import numpy as np
import ml_dtypes

import concourse.bass as bass
import concourse.tile as tile
from concourse import bacc
from concourse import masks
from concourse import mybir
from concourse.alu_op_type import AluOpType
from concourse.bass_utils import run_bass_kernel_spmd

B, S, H, R = 8, 2048, 768, 768
N_CORES = 8
P = 128          # SBUF partitions
NC_ = 512        # matmul moving free dim (one PSUM bank of fp32)
HT = H // P      # 6  k-tiles over H
RT = R // P      # 6  k-tiles over R
IT = S // P      # 16 output row tiles
SC = S // NC_    # 4  512-wide column chunks
TPC = NC_ // P   # 4  row tiles per chunk
PAIRS = HT // 2  # 3  DoubleRow k-tile pairs per 768 contraction

F32 = mybir.dt.float32
F32R = mybir.dt.float32r
BF16 = mybir.dt.bfloat16
F8 = mybir.dt.float8e4
DR = mybir.MatmulPerfMode.DoubleRow

NPF8 = ml_dtypes.float8_e4m3


def build_nc(reps=1):
    nc = bacc.Bacc("TRN2", target_bir_lowering=False, debug=False,
                   num_devices=N_CORES)

    xh_d = nc.dram_tensor("xh", [P, HT, S], F8, kind="ExternalInput")
    xl_d = nc.dram_tensor("xl", [P, HT, S], F8, kind="ExternalInput")
    ph_d = nc.dram_tensor("ph", [P, HT, R], F8, kind="ExternalInput")
    pl_d = nc.dram_tensor("pl", [P, HT, R], F8, kind="ExternalInput")
    # partition-major output layout: element (p, it, s) = d2[it*128+p, s].
    # Lets consecutive row-tile blocks share one DMA (host untiles).
    out_d = nc.dram_tensor("out", [P, IT, S], BF16, kind="ExternalOutput")

    with tile.TileContext(nc) as tc:
        with tc.tile_pool(name="persist", bufs=1) as sb, \
             tc.tile_pool(name="stg", bufs=4) as stg, \
             tc.tile_pool(name="pmm", bufs=3, space="PSUM") as pmm, \
             tc.tile_pool(name="pd", bufs=5, space="PSUM") as pdp:

            # input + qq buffers are double-buffered by rep parity so the
            # next rep's projection phase (and its input DMAs) can overlap
            # this rep's Gram waves without write-after-read hazards.
            xh_sb = [sb.tile([P, HT, S], F8, name=f"xh{i}", tag=f"xh{i}")
                     for i in range(2)]
            xl_sb = [sb.tile([P, HT, S], F8, name=f"xl{i}", tag=f"xl{i}")
                     for i in range(2)]
            ph_sb = [sb.tile([P, HT, R], F8, name=f"ph{i}", tag=f"ph{i}")
                     for i in range(2)]
            pl_sb = [sb.tile([P, HT, R], F8, name=f"pl{i}", tag=f"pl{i}")
                     for i in range(2)]
            qq_sb = [sb.tile([P, RT, S], F8, name=f"qq{i}", tag=f"qq{i}")
                     for i in range(2)]
            sqj = sb.tile([P, S], F32, name="sqj", tag="sqj")
            sqcol = sb.tile([P, IT], F32, name="sqcol", tag="sqcol")
            ident4 = sb.tile([P, NC_], F32, name="ident4", tag="id4")
            onesf = sb.tile([P, P], F32, name="onesf", tag="onesf")
            onesr = sb.tile([P, P], F32R, name="onesr", tag="onesr")

            for k in range(TPC):
                masks.make_identity(nc, ident4[:, k * P:(k + 1) * P])
            nc.vector.memset(onesf[:], 1.0)
            nc.vector.tensor_copy(onesr[:], onesf[:])

            def emit_loads(par):
                """Input DMAs for the buffers of rep parity `par`."""
                nc.sync.dma_start(ph_sb[par][:], ph_d[:, :, :])
                nc.sync.dma_start(pl_sb[par][:], pl_d[:, :, :])
                for c in range(SC):
                    cs = slice(c * NC_, (c + 1) * NC_)
                    nc.sync.dma_start(xh_sb[par][:, :, cs], xh_d[:, :, cs])
                    nc.sync.dma_start(xl_sb[par][:, :, cs], xl_d[:, :, cs])

            def emit_body(par):
                xh, xl = xh_sb[par], xl_sb[par]
                ph, pl = ph_sb[par], pl_sb[par]
                qq = qq_sb[par]

                # Epilogue engine schedule, per unit index mod 16. GPSIMD
                # cannot read PSUM, so PSUM-input stt lives on DVE; for
                # "type-B" slots ACT first evacuates -2*pd to SBUF so Pool
                # can add sqj, and the cheap bf16 relu goes to DVE.
                TYPEB = frozenset((1, 4, 7, 10, 13))
                RELU_A = frozenset((0, 2, 6, 9, 12, 14))
                unit_idx = [0]

                def emit_mm(it, jc):
                    """Gram matmul group for one [128, 512] tile."""
                    js = slice(jc * NC_, (jc + 1) * NC_)
                    pd = pdp.tile([P, NC_], F32, name="pd", tag="pd")
                    for p in range(PAIRS):
                        nc.tensor.matmul(
                            pd[:],
                            qq[:, 2 * p:2 * p + 2, it * P:(it + 1) * P],
                            qq[:, 2 * p:2 * p + 2, js],
                            start=(p == 0), stop=(p == PAIRS - 1),
                            perf_mode=DR)
                    return pd

                def emit_epilogue(it, jc, pd, dst):
                    """relu(-2*pd + sq_j + sq_i) -> bf16 into dst AP."""
                    js = slice(jc * NC_, (jc + 1) * NC_)
                    u = unit_idx[0] % 16
                    unit_idx[0] += 1
                    st = stg.tile([P, NC_], F32, name="st", tag="st",
                                  bufs=4)
                    if u in TYPEB:
                        pb = stg.tile([P, NC_], F32, name="pb", tag="pb",
                                      bufs=3)
                        nc.scalar.activation(
                            pb[:], pd[:], mybir.ActivationFunctionType.Copy,
                            bias=0.0, scale=-2.0)
                        nc.gpsimd.tensor_tensor(st[:], pb[:], sqj[:, js],
                                                AluOpType.add)
                        nc.vector.tensor_scalar(
                            dst, st[:], sqcol[:, it:it + 1], 0.0,
                            AluOpType.add, AluOpType.max)
                    else:
                        nc.vector.scalar_tensor_tensor(
                            st[:], pd[:], -2.0, sqj[:, js],
                            AluOpType.mult, AluOpType.add)
                        if u in RELU_A:
                            nc.scalar.activation(
                                dst, st[:],
                                mybir.ActivationFunctionType.Relu,
                                bias=sqcol[:, it:it + 1], scale=1.0)
                        else:
                            nc.gpsimd.tensor_scalar(
                                dst, st[:], sqcol[:, it:it + 1], 0.0,
                                AluOpType.add, AluOpType.max)

                def emit_proj_chunk(c):
                    """t' for columns chunk c -> quantized qq chunk."""
                    cs = slice(c * NC_, (c + 1) * NC_)
                    for rt in range(RT):
                        pt = pmm.tile([P, NC_], F32, name="pt", tag="pt")
                        first = True
                        for pj, xx in ((ph, xh), (ph, xl), (pl, xh)):
                            for p in range(PAIRS):
                                nc.tensor.matmul(
                                    pt[:],
                                    pj[:, 2 * p:2 * p + 2,
                                       rt * P:(rt + 1) * P],
                                    xx[:, 2 * p:2 * p + 2, cs],
                                    start=first,
                                    stop=(pj is pl and p == PAIRS - 1),
                                    perf_mode=DR)
                                first = False
                        nc.scalar.copy(qq[:, rt, cs], pt[:])

                def emit_wave(c):
                    """All Gram tiles with max(row_chunk, col_chunk)==c.

                    The 4 diagonal-containing tiles go first; sq for
                    chunk c is extracted from their PSUM (fused mask +
                    free-axis reduce) before any wave-c epilogue runs.
                    Row tiles of chunk c accumulate into one bf16 strip
                    so each row needs a single output DMA.
                    """
                    cs = slice(c * NC_, (c + 1) * NC_)
                    diag_pds = []
                    for k in range(TPC):
                        it = c * TPC + k
                        diag_pds.append((it, emit_mm(it, c)))
                    dm = stg.tile([P, NC_], F32R, name="dm", tag="dm",
                                  bufs=2)
                    for k, (it, pd) in enumerate(diag_pds):
                        ks = slice(k * P, (k + 1) * P)
                        nc.vector.tensor_mul(dm[:, ks], pd[:, ks],
                                             ident4[:, ks])
                    sq_ps = pmm.tile([P, NC_], F32, name="sqps", tag="pt")
                    nc.tensor.matmul(sq_ps[:], onesr[:], dm[:],
                                     start=True, stop=True)
                    nc.scalar.copy(sqj[:, cs], sq_ps[:])
                    for k, (it, pd) in enumerate(diag_pds):
                        ks = slice(k * P, (k + 1) * P)
                        nc.vector.tensor_reduce(
                            sqcol[:, it:it + 1], dm[:, ks],
                            axis=mybir.AxisListType.X, op=AluOpType.add)
                    w = (c + 1) * NC_
                    for k in range(TPC):
                        it = c * TPC + k
                        strip = stg.tile([P, S], BF16, name="rs", tag="rs",
                                         bufs=5)
                        # diag epilogue first: frees its PSUM bank before
                        # the row's remaining matmuls need banks
                        emit_epilogue(it, c, diag_pds[k][1],
                                      strip[:, c * NC_:(c + 1) * NC_])
                        for jc in range(c):
                            emit_epilogue(it, jc, emit_mm(it, jc),
                                          strip[:, jc * NC_:(jc + 1) * NC_])
                        nc.sync.dma_start(out_d[:, it, 0:w], strip[:, 0:w])
                    # column tiles (rows from earlier chunks), DMA'd in
                    # batches of 4 row-tiles via the partition-major layout
                    for it0 in range(0, c * TPC, TPC):
                        n = min(TPC, c * TPC - it0)
                        cb = stg.tile([P, TPC, NC_], BF16, name="cb",
                                      tag="cb", bufs=3)
                        for k in range(n):
                            emit_epilogue(it0 + k, c, emit_mm(it0 + k, c),
                                          cb[:, k, :])
                        nc.sync.dma_start(out_d[:, it0:it0 + n, cs],
                                          cb[:, 0:n, :])

                # chunk-pipelined schedule: wave c is emitted after
                # projection chunk c+1 so the fp8 quantize of chunk c has
                # drained before PE reaches wave c's matmuls. The next
                # rep's input DMAs are emitted once this rep's projection
                # has consumed its inputs, so their transfers overlap the
                # Gram waves.
                emit_proj_chunk(0)
                for c in range(SC):
                    if c + 1 < SC:
                        emit_proj_chunk(c + 1)
                    if c == 0:
                        emit_loads(1 - par)
                    emit_wave(c)

            emit_loads(0)
            for r in range(reps):
                emit_body(r % 2)

    nc.finalize()
    return nc


_NC_CACHE = {}


def get_nc(reps=1):
    key = reps
    if key not in _NC_CACHE:
        _NC_CACHE[key] = build_nc(reps)
    return _NC_CACHE[key]


def _split8(a):
    """hi/lo fp8e4 residual split of a float32 array."""
    hi = a.astype(NPF8)
    lo = (a - hi.astype(np.float32)).astype(NPF8)
    return hi, lo


def _pack(a8):
    """[H, N] -> [128, HT, N] partition-major tiling."""
    n = a8.shape[1]
    return np.ascontiguousarray(
        a8.reshape(HT, P, n).transpose(1, 0, 2))


def make_in_maps(batch, proj):
    ph, pl = _split8(np.ascontiguousarray(proj, dtype=np.float32))
    ph, pl = _pack(ph), _pack(pl)
    maps = []
    for b in range(B):
        xT = np.ascontiguousarray(batch[b].T, dtype=np.float32)
        xh, xl = _split8(xT)
        maps.append({"xh": _pack(xh), "xl": _pack(xl), "ph": ph, "pl": pl})
    return maps


def kernel(batch, proj):
    assert batch.shape == (B, S, H) and proj.shape == (H, R)
    nc = get_nc()
    in_maps = make_in_maps(batch, proj)
    res = run_bass_kernel_spmd(nc, in_maps, core_ids=list(range(N_CORES)))
    out = np.stack(
        [np.asarray(res.results[b]["out"]).transpose(1, 0, 2).reshape(S, S)
         for b in range(B)], axis=0)
    return out.astype(np.float32)


# revision 30
# speedup vs baseline: 1.1826x; 1.0010x over previous
"""Trainium2 Bass kernel for nn_DistanceProbe.

Computes, for batch [B=8, S=2048, H=768] and proj [H=768, R=768]:
    t  = batch @ proj                      # [B, S, R]
    d2 = relu(||t_i||^2 + ||t_j||^2 - 2 t_i . t_j)   # [B, S, S]

Sharding: data-parallel over B across the 8 NeuronCores (one batch
element per core).

Numerics/performance strategy (validated vs reference in fp8 numpy sim,
max-abs/scale err ~1.3e-2 < 2e-2 gate):
  * Host splits each input into hi/lo fp8e4 pairs: x ~= xh + xl,
    proj ~= ph + pl (residual quantization, ~0.2% relative).
  * Projection t' = xh@ph + xl@ph + xh@pl on PE as fp8e4 DoubleRow
    matmuls (0.5 cyc/row: 2x bf16 rate). Dropped xl@pl term ~0.1%.
  * t' is quantized to fp8e4 (q) by the ACT engine; the SxS Gram matrix
    dots = q.T q runs as fp8e4 DoubleRow matmuls.
  * sq_i = dots_ii is read out of the diagonal-containing Gram tiles
    (identity mask + free-axis reduce on DVE; ones-matmul rebroadcast
    for the row form) => bitwise-consistent with dots, so the relu
    clamp and the zero diagonal are exact in fp8 arithmetic.
  * Epilogue relu(-2*dots + sq_j + sq_i) is two elementwise passes
    (scalar_tensor_tensor; then +bias relu) distributed across
    DVE/ACT/Pool; output written bf16 (lossless host upcast to f32).
  * Emission is chunk-pipelined: Gram wave c is interleaved one chunk
    behind the projection matmuls so every engine streams.

`reps` repeats the whole body inside one NEFF (used by test.py to
measure steady-state HW time by differencing two rep counts).
"""

import numpy as np
import ml_dtypes

import concourse.bass as bass
import concourse.tile as tile
from concourse import bacc
from concourse import masks
from concourse import mybir
from concourse.alu_op_type import AluOpType
from concourse.bass_utils import run_bass_kernel_spmd

B, S, H, R = 8, 2048, 768, 768
N_CORES = 8
P = 128          # SBUF partitions
NC_ = 512        # matmul moving free dim (one PSUM bank of fp32)
HT = H // P      # 6  k-tiles over H
RT = R // P      # 6  k-tiles over R
IT = S // P      # 16 output row tiles
SC = S // NC_    # 4  512-wide column chunks
TPC = NC_ // P   # 4  row tiles per chunk
PAIRS = HT // 2  # 3  DoubleRow k-tile pairs per 768 contraction

F32 = mybir.dt.float32
F32R = mybir.dt.float32r
BF16 = mybir.dt.bfloat16
F8 = mybir.dt.float8e4
DR = mybir.MatmulPerfMode.DoubleRow

NPF8 = ml_dtypes.float8_e4m3


def build_nc(reps=1):
    nc = bacc.Bacc("TRN2", target_bir_lowering=False, debug=False,
                   num_devices=N_CORES)

    xh_d = nc.dram_tensor("xh", [P, HT, S], F8, kind="ExternalInput")
    xl_d = nc.dram_tensor("xl", [P, HT, S], F8, kind="ExternalInput")
    ph_d = nc.dram_tensor("ph", [P, HT, R], F8, kind="ExternalInput")
    pl_d = nc.dram_tensor("pl", [P, HT, R], F8, kind="ExternalInput")
    # partition-major output layout: element (p, it, s) = d2[it*128+p, s].
    # Lets consecutive row-tile blocks share one DMA (host untiles).
    out_d = nc.dram_tensor("out", [P, IT, S], BF16, kind="ExternalOutput")

    with tile.TileContext(nc) as tc:
        with tc.tile_pool(name="persist", bufs=1) as sb, \
             tc.tile_pool(name="stg", bufs=4) as stg, \
             tc.tile_pool(name="pmm", bufs=3, space="PSUM") as pmm, \
             tc.tile_pool(name="pd", bufs=5, space="PSUM") as pdp:

            # input + qq buffers are double-buffered by rep parity so the
            # next rep's projection phase (and its input DMAs) can overlap
            # this rep's Gram waves without write-after-read hazards.
            xh_sb = [sb.tile([P, HT, S], F8, name=f"xh{i}", tag=f"xh{i}")
                     for i in range(2)]
            xl_sb = [sb.tile([P, HT, S], F8, name=f"xl{i}", tag=f"xl{i}")
                     for i in range(2)]
            ph_sb = [sb.tile([P, HT, R], F8, name=f"ph{i}", tag=f"ph{i}")
                     for i in range(2)]
            pl_sb = [sb.tile([P, HT, R], F8, name=f"pl{i}", tag=f"pl{i}")
                     for i in range(2)]
            qq_sb = [sb.tile([P, RT, S], F8, name=f"qq{i}", tag=f"qq{i}")
                     for i in range(2)]
            sqj = sb.tile([P, S], F32, name="sqj", tag="sqj")
            sqcol = sb.tile([P, IT], F32, name="sqcol", tag="sqcol")
            ident4 = sb.tile([P, NC_], F32, name="ident4", tag="id4")
            onesf = sb.tile([P, P], F32, name="onesf", tag="onesf")
            onesr = sb.tile([P, P], F32R, name="onesr", tag="onesr")

            for k in range(TPC):
                masks.make_identity(nc, ident4[:, k * P:(k + 1) * P])
            nc.vector.memset(onesf[:], 1.0)
            nc.vector.tensor_copy(onesr[:], onesf[:])

            def emit_loads(par):
                """Input DMAs for the buffers of rep parity `par`."""
                nc.sync.dma_start(ph_sb[par][:], ph_d[:, :, :])
                nc.sync.dma_start(pl_sb[par][:], pl_d[:, :, :])
                for c in range(SC):
                    cs = slice(c * NC_, (c + 1) * NC_)
                    nc.sync.dma_start(xh_sb[par][:, :, cs], xh_d[:, :, cs])
                    nc.sync.dma_start(xl_sb[par][:, :, cs], xl_d[:, :, cs])

            def emit_body(par):
                xh, xl = xh_sb[par], xl_sb[par]
                ph, pl = ph_sb[par], pl_sb[par]
                qq = qq_sb[par]

                # Epilogue engine schedule, per unit index mod 16. GPSIMD
                # cannot read PSUM, so PSUM-input stt lives on DVE; for
                # "type-B" slots ACT first evacuates -2*pd to SBUF so Pool
                # can add sqj, and the cheap bf16 relu goes to DVE.
                TYPEB = frozenset((1, 4, 7, 10, 13))
                RELU_A = frozenset((0, 2, 6, 9, 12, 14))
                unit_idx = [0]

                def emit_mm(it, jc):
                    """Gram matmul group for one [128, 512] tile."""
                    js = slice(jc * NC_, (jc + 1) * NC_)
                    pd = pdp.tile([P, NC_], F32, name="pd", tag="pd")
                    for p in range(PAIRS):
                        nc.tensor.matmul(
                            pd[:],
                            qq[:, 2 * p:2 * p + 2, it * P:(it + 1) * P],
                            qq[:, 2 * p:2 * p + 2, js],
                            start=(p == 0), stop=(p == PAIRS - 1),
                            perf_mode=DR)
                    return pd

                def emit_epilogue(it, jc, pd, dst):
                    """relu(-2*pd + sq_j + sq_i) -> bf16 into dst AP."""
                    js = slice(jc * NC_, (jc + 1) * NC_)
                    u = unit_idx[0] % 16
                    unit_idx[0] += 1
                    st = stg.tile([P, NC_], BF16, name="st", tag="st",
                                  bufs=4)
                    if u in TYPEB:
                        pb = stg.tile([P, NC_], F32, name="pb", tag="pb",
                                      bufs=3)
                        nc.scalar.activation(
                            pb[:], pd[:], mybir.ActivationFunctionType.Copy,
                            bias=0.0, scale=-2.0)
                        nc.gpsimd.tensor_tensor(st[:], pb[:], sqj[:, js],
                                                AluOpType.add)
                        nc.vector.tensor_scalar(
                            dst, st[:], sqcol[:, it:it + 1], 0.0,
                            AluOpType.add, AluOpType.max)
                    else:
                        nc.vector.scalar_tensor_tensor(
                            st[:], pd[:], -2.0, sqj[:, js],
                            AluOpType.mult, AluOpType.add)
                        if u in RELU_A:
                            nc.scalar.activation(
                                dst, st[:],
                                mybir.ActivationFunctionType.Relu,
                                bias=sqcol[:, it:it + 1], scale=1.0)
                        else:
                            nc.gpsimd.tensor_scalar(
                                dst, st[:], sqcol[:, it:it + 1], 0.0,
                                AluOpType.add, AluOpType.max)

                def emit_proj_chunk(c):
                    """t' for columns chunk c -> quantized qq chunk."""
                    cs = slice(c * NC_, (c + 1) * NC_)
                    for rt in range(RT):
                        pt = pmm.tile([P, NC_], F32, name="pt", tag="pt")
                        first = True
                        for pj, xx in ((ph, xh), (ph, xl), (pl, xh)):
                            for p in range(PAIRS):
                                nc.tensor.matmul(
                                    pt[:],
                                    pj[:, 2 * p:2 * p + 2,
                                       rt * P:(rt + 1) * P],
                                    xx[:, 2 * p:2 * p + 2, cs],
                                    start=first,
                                    stop=(pj is pl and p == PAIRS - 1),
                                    perf_mode=DR)
                                first = False
                        nc.scalar.copy(qq[:, rt, cs], pt[:])

                def emit_wave(c):
                    """All Gram tiles with max(row_chunk, col_chunk)==c.

                    The 4 diagonal-containing tiles go first; sq for
                    chunk c is extracted from their PSUM (fused mask +
                    free-axis reduce) before any wave-c epilogue runs.
                    Row tiles of chunk c accumulate into one bf16 strip
                    so each row needs a single output DMA.
                    """
                    cs = slice(c * NC_, (c + 1) * NC_)
                    diag_pds = []
                    for k in range(TPC):
                        it = c * TPC + k
                        diag_pds.append((it, emit_mm(it, c)))
                    dm = stg.tile([P, NC_], F32R, name="dm", tag="dm",
                                  bufs=2)
                    for k, (it, pd) in enumerate(diag_pds):
                        ks = slice(k * P, (k + 1) * P)
                        nc.vector.tensor_mul(dm[:, ks], pd[:, ks],
                                             ident4[:, ks])
                    sq_ps = pmm.tile([P, NC_], F32, name="sqps", tag="pt")
                    nc.tensor.matmul(sq_ps[:], onesr[:], dm[:],
                                     start=True, stop=True)
                    nc.scalar.copy(sqj[:, cs], sq_ps[:])
                    for k, (it, pd) in enumerate(diag_pds):
                        ks = slice(k * P, (k + 1) * P)
                        nc.vector.tensor_reduce(
                            sqcol[:, it:it + 1], dm[:, ks],
                            axis=mybir.AxisListType.X, op=AluOpType.add)
                    w = (c + 1) * NC_
                    for k in range(TPC):
                        it = c * TPC + k
                        strip = stg.tile([P, S], BF16, name="rs", tag="rs",
                                         bufs=5)
                        # diag epilogue first: frees its PSUM bank before
                        # the row's remaining matmuls need banks
                        emit_epilogue(it, c, diag_pds[k][1],
                                      strip[:, c * NC_:(c + 1) * NC_])
                        for jc in range(c):
                            emit_epilogue(it, jc, emit_mm(it, jc),
                                          strip[:, jc * NC_:(jc + 1) * NC_])
                        nc.sync.dma_start(out_d[:, it, 0:w], strip[:, 0:w])
                    # column tiles (rows from earlier chunks), DMA'd in
                    # batches of 4 row-tiles via the partition-major layout
                    for it0 in range(0, c * TPC, TPC):
                        n = min(TPC, c * TPC - it0)
                        cb = stg.tile([P, TPC, NC_], BF16, name="cb",
                                      tag="cb", bufs=3)
                        for k in range(n):
                            emit_epilogue(it0 + k, c, emit_mm(it0 + k, c),
                                          cb[:, k, :])
                        nc.sync.dma_start(out_d[:, it0:it0 + n, cs],
                                          cb[:, 0:n, :])

                # chunk-pipelined schedule: wave c is emitted after
                # projection chunk c+1 so the fp8 quantize of chunk c has
                # drained before PE reaches wave c's matmuls. The next
                # rep's input DMAs are emitted once this rep's projection
                # has consumed its inputs, so their transfers overlap the
                # Gram waves.
                emit_proj_chunk(0)
                for c in range(SC):
                    if c + 1 < SC:
                        emit_proj_chunk(c + 1)
                    if c == 0:
                        emit_loads(1 - par)
                    emit_wave(c)

            emit_loads(0)
            for r in range(reps):
                emit_body(r % 2)

    nc.finalize()
    return nc


_NC_CACHE = {}


def get_nc(reps=1):
    key = reps
    if key not in _NC_CACHE:
        _NC_CACHE[key] = build_nc(reps)
    return _NC_CACHE[key]


def _split8(a):
    """hi/lo fp8e4 residual split of a float32 array."""
    hi = a.astype(NPF8)
    lo = (a - hi.astype(np.float32)).astype(NPF8)
    return hi, lo


def _pack(a8):
    """[H, N] -> [128, HT, N] partition-major tiling."""
    n = a8.shape[1]
    return np.ascontiguousarray(
        a8.reshape(HT, P, n).transpose(1, 0, 2))


def make_in_maps(batch, proj):
    ph, pl = _split8(np.ascontiguousarray(proj, dtype=np.float32))
    ph, pl = _pack(ph), _pack(pl)
    maps = []
    for b in range(B):
        xT = np.ascontiguousarray(batch[b].T, dtype=np.float32)
        xh, xl = _split8(xT)
        maps.append({"xh": _pack(xh), "xl": _pack(xl), "ph": ph, "pl": pl})
    return maps


def kernel(batch, proj):
    assert batch.shape == (B, S, H) and proj.shape == (H, R)
    nc = get_nc()
    in_maps = make_in_maps(batch, proj)
    res = run_bass_kernel_spmd(nc, in_maps, core_ids=list(range(N_CORES)))
    out = np.stack(
        [np.asarray(res.results[b]["out"]).transpose(1, 0, 2).reshape(S, S)
         for b in range(B)], axis=0)
    return out.astype(np.float32)
